# revision 8
# baseline (speedup 1.0000x reference)
"""BiLSTM-CRF NLL loss on 8 Trainium2 NeuronCores.

Sharding: core c owns sequences [4c, 4c+4); each core runs BOTH LSTM
directions and the full CRF for its 4 sequences. No collectives; host sums
8 per-core partials.

Recurrence: TWO phase-shifted chains per core, chain ch owns seqs
{2ch, 2ch+1} and carries BOTH directions in merged instructions. Per
chain-step: one PSUM tile G [128, 64] (cols = d*32 + m*2 + s'), one
accumulation group: bias-inject identity mm (start=True) + 32 x-mms +
64 h-mms (stop on last). sigma-trick: g-gate rows of W/b pre-scaled by 2 so
ONE Sigmoid activation covers all 16 gate chunks (tanh(g) = 2*sigma(2g)-1);
cell update is 4 fused DVE ops (c = t1 + 2*i*s - i), then tanh(c) Act and
the h-mul DVE writes both directions' h slots via a strided AP. This cuts
the per-step serial chain to PE -> Act -> DVE -> Act -> DVE with two
independent chains pipelined to hide the cross-engine latency.

Weights fp8e4m3, x/h matmuls fp8 DoubleRow. h stored fp8 unit-major.

CRF partition function: exp-domain, two-sided (alpha ascends, beta descends,
meet at K=127), bf16 chain operands. Gold-path score via one-hot matmuls.
Loss exits as [1,4] per-core partials.

Self-contained: hardcodes all shapes; only needs numpy + concourse (+ml_dtypes).
"""
import numpy as np
import ml_dtypes

import concourse.bass as bass
import concourse.bacc as bacc
import concourse.tile as tile
from concourse import mybir
from concourse.tile_rust import add_dep_helper
from concourse.bass_utils import run_bass_kernel_spmd

F32 = mybir.dt.float32
FP8 = mybir.dt.float8e4
BF16 = mybir.dt.bfloat16
I32 = mybir.dt.int32
AF = mybir.ActivationFunctionType
ALU = mybir.AluOpType

B, S, E, H, T, V = 32, 256, 256, 512, 45, 50000
NS = 4                 # seqs per core
N = S * NS             # 1024 emission cols, n = 4t+s
NCH = 16               # gate chunks (2048/128)
HC = 4                 # h chunks (512/128)
SW = HC * NS           # state cols per step = 16 (all 4 seqs)
NB_T = 4               # transform n-blocks (of 64 steps = 256 cols each)
TBLK = S // NB_T       # 64 steps per transform block
LN45 = float(np.log(45.0))
DSTRIDE = (S + 1) * SW  # 4112: cols per direction in hsT_all

_cached = {}


def _build(stop_after=None):
    lv = {"xf": 1, "rec": 2, "em": 3, "crf": 4, None: 5}[stop_after]
    nc = bacc.Bacc("TRN2", target_bir_lowering=False, debug=False, num_devices=8)

    d = {}
    d["emb"] = nc.dram_tensor("emb", [V, E], BF16, kind="ExternalInput")
    d["xidx"] = nc.dram_tensor("xidx", [128, 8], I32, kind="ExternalInput")
    d["wihf"] = nc.dram_tensor("wihf", [128, 32 * 128], FP8, kind="ExternalInput")
    d["wihb"] = nc.dram_tensor("wihb", [128, 32 * 128], FP8, kind="ExternalInput")
    d["whhf"] = nc.dram_tensor("whhf", [128, 64 * 128], FP8, kind="ExternalInput")
    d["whhb"] = nc.dram_tensor("whhb", [128, 64 * 128], FP8, kind="ExternalInput")
    d["biasbc"] = nc.dram_tensor("biasbc", [128, 64], BF16, kind="ExternalInput")
    d["linT"] = nc.dram_tensor("linT", [128, 8 * T], BF16, kind="ExternalInput")
    d["linb"] = nc.dram_tensor("linb", [T, 1], F32, kind="ExternalInput")
    d["id128"] = nc.dram_tensor("id128", [128, 128], F32, kind="ExternalInput")
    d["idbf"] = nc.dram_tensor("idbf", [128, 128], BF16, kind="ExternalInput")
    d["trans"] = nc.dram_tensor("trans", [T, T], F32, kind="ExternalInput")
    d["stend"] = nc.dram_tensor("stend", [T, 2], F32, kind="ExternalInput")
    d["oh"] = nc.dram_tensor("oh", [T, N], F32, kind="ExternalInput")
    d["oh2"] = nc.dram_tensor("oh2", [T, N], F32, kind="ExternalInput")
    d_loss = nc.dram_tensor("loss", [1, NS], F32, kind="ExternalOutput")

    with tile.TileContext(nc) as tc:
        with tc.tile_pool(name="persist", bufs=1) as pp, \
             tc.tile_pool(name="gxp", bufs=1) as gxp:
            # persistent weights / tables
            wih = {0: pp.tile([128, 32 * 128], FP8, tag="wihf", name="wihf"),
                   1: pp.tile([128, 32 * 128], FP8, tag="wihb", name="wihb")}
            whh = {0: pp.tile([128, 64 * 128], FP8, tag="whhf", name="whhf"),
                   1: pp.tile([128, 64 * 128], FP8, tag="whhb", name="whhb")}
            biasbc = pp.tile([128, 64], BF16, tag="biasbc")
            ones1 = pp.tile([1, NS], F32, tag="ones1")
            id128 = pp.tile([128, 128], F32, tag="id128")
            idbf = pp.tile([128, 128], BF16, tag="idbf")
            xidx = pp.tile([128, 8], I32, tag="xidx")
            linT = pp.tile([128, 8 * T], BF16, tag="linT")
            nc.sync.dma_start(out=xidx[:], in_=d["xidx"][:])
            nc.sync.dma_start(out=id128[:], in_=d["id128"][:])
            nc.sync.dma_start(out=idbf[:], in_=d["idbf"][:])
            nc.sync.dma_start(out=biasbc[:], in_=d["biasbc"][:])
            nc.vector.memset(ones1[:], 1.0)

            # XT block tiles: [nb] -> [128, 2 ec x 256 n] fp8
            xt = {nb: gxp.tile([128, 2 * TBLK * NS], FP8, tag=f"xt{nb}", name=f"xt{nb}")
                  for nb in range(NB_T)}
            # h state, both dirs in one tile (unit-major):
            # col = d*DSTRIDE + 16*slot + 4*k + s ; slot S = zeros (h0).
            # BOTH directions write slot u at recurrence step u: the bwd
            # direction's h (position S-1-u) is stored TIME-REVERSED, so
            # h-mm reads and the h-write are uniform across dirs. The
            # reversal is undone in the emissions phase via a reversed AP.
            hsT = pp.tile([128, 2 * DSTRIDE], FP8, tag="hsT", name="hsT")
            nc.vector.memset(hsT[:, SW * S: SW * S + SW], 0.0)
            nc.vector.memset(hsT[:, DSTRIDE + SW * S: DSTRIDE + SW * S + SW], 0.0)

            # ---------- phase 0: gather + transpose -> XT ----------
            with tc.tile_pool(name="gat", bufs=3) as gp, \
                 tc.tile_pool(name="ps_tp", bufs=4, space="PSUM") as ps_tp:
                for b in range(8):
                    X = gp.tile([128, E], BF16, tag="X")
                    nc.gpsimd.indirect_dma_start(
                        out=X[:],
                        out_offset=None,
                        in_=d["emb"][:],
                        in_offset=bass.IndirectOffsetOnAxis(ap=xidx[:, b:b + 1], axis=0),
                    )
                    nb, off = b // 2, (b % 2) * 128
                    for ec in range(2):
                        tp = ps_tp.tile([128, 128], BF16, tag="tp")
                        nc.tensor.transpose(tp[:], X[:, 128 * ec: 128 * ec + 128], idbf[:])
                        nc.vector.tensor_copy(
                            xt[nb][:, TBLK * NS * ec + off: TBLK * NS * ec + off + 128],
                            tp[:])

            # weight DMAs after the gathers so they share the DMA engines
            nc.sync.dma_start(out=wih[0][:], in_=d["wihf"][:])
            nc.sync.dma_start(out=wih[1][:], in_=d["wihb"][:])
            nc.sync.dma_start(out=whh[0][:], in_=d["whhf"][:])
            nc.sync.dma_start(out=whh[1][:], in_=d["whhb"][:])
            nc.sync.dma_start(out=linT[:], in_=d["linT"][:])

            # ---------- recurrence ----------
            if lv == 1:
                probe = pp.tile([1, NS], F32, tag="probe")
                nc.vector.tensor_copy(probe[:], xt[0][0:1, 0:NS])
                nc.sync.dma_start(out=d_loss[:], in_=probe[:])
            if lv >= 2:
                with tc.tile_pool(name="rec0", bufs=6) as rp0, \
                     tc.tile_pool(name="rec1", bufs=6) as rp1, \
                     tc.tile_pool(name="psg0", bufs=2, space="PSUM") as pg0, \
                     tc.tile_pool(name="psg1", bufs=2, space="PSUM") as pg1:
                    rp = [rp0, rp1]
                    pg = [pg0, pg1]
                    cprev = [None, None]
                    for ch in (0, 1):
                        cinit = rp[ch].tile([128, 16], BF16, tag="c")
                        nc.vector.memset(cinit[:], 0.0)
                        cprev[ch] = cinit

                    DR = mybir.MatmulPerfMode.DoubleRow
                    xtv = {nb: xt[nb].rearrange("p (e c) -> p e c", e=2)
                           for nb in range(NB_T)}
                    hv = hsT.rearrange("p (a t k s) -> p a t k s",
                                       a=2, t=S + 1, k=HC, s=NS)

                    def rstep(ch, u):
                        # chain ch owns seqs {2ch, 2ch+1}, both directions.
                        # Both dirs write slot u, read slot u-1 (u=0: slot S
                        # = zeros); bwd h is stored time-reversed.
                        slot_r = S if u == 0 else u - 1
                        col = {0: u, 1: S - 1 - u}   # xt timestep per dir

                        G = pg[ch].tile([128, 64], F32, tag="G")
                        # one accumulation group per G tile: bias-inject
                        # (start=True zeroes the region) -> x-mms -> h-mms
                        # (stop=True on the very last; at u=0 there are no
                        # h-mms so the last x-mm stops). add_dep_helper pins
                        # start-first / stop-last against scheduler
                        # reordering. Phase A (inject + x) has no h dep and
                        # runs in the previous step's tail.
                        mms = []
                        mm = nc.tensor.matmul(G[:], idbf[:], biasbc[:],
                                              start=True, stop=False)
                        mms.append(mm)
                        for dd in (0, 1):
                            nb = col[dd] // TBLK
                            j = col[dd] % TBLK
                            base_x = NS * j + 2 * ch
                            for m in range(NCH):
                                sl = G[:, dd * 32 + 2 * m: dd * 32 + 2 * m + 2]
                                wpair = wih[dd][:, 2 * m * 128:(2 * m + 2) * 128]
                                mm = nc.tensor.matmul(
                                    sl,
                                    wpair.rearrange("p (c f) -> p c f", c=2),
                                    xtv[nb][:, :, base_x: base_x + 2],
                                    start=False,
                                    stop=(u == 0 and dd == 1 and m == NCH - 1),
                                    perf_mode=DR)
                                mms.append(mm)
                        if u > 0:
                            for dd in (0, 1):
                                for m in range(NCH):
                                    sl = G[:, dd * 32 + 2 * m:
                                           dd * 32 + 2 * m + 2]
                                    for kp in range(HC // 2):
                                        hpair = whh[dd][
                                            :, (4 * m + 2 * kp) * 128:
                                            (4 * m + 2 * kp + 2) * 128]
                                        rhs = hv[:, dd, slot_r,
                                                 2 * kp: 2 * kp + 2,
                                                 2 * ch: 2 * ch + 2]
                                        mm = nc.tensor.matmul(
                                            sl,
                                            hpair.rearrange(
                                                "p (c f) -> p c f", c=2),
                                            rhs,
                                            start=False,
                                            stop=(dd == 1 and m == NCH - 1
                                                  and kp == HC // 2 - 1),
                                            perf_mode=DR)
                                        mms.append(mm)
                        first, last = mms[0], mms[-1]
                        for mm in mms[1:]:
                            add_dep_helper(mm.ins, first.ins, sync=False,
                                           reason="group start first")
                        for mm in mms[:-1]:
                            add_dep_helper(last.ins, mm.ins, sync=False,
                                           reason="group stop last")

                        # sigma over ALL 64 gate cols (g rows pre-scaled x2)
                        SG = rp[ch].tile([128, 64], BF16, tag="SG")
                        nc.scalar.activation(SG[:], G[:], AF.Sigmoid)
                        SGv = SG.rearrange("p (a m s) -> p a m s", a=2, m=NCH)
                        ihat = SGv[:, :, 0:4, :]
                        fhat = SGv[:, :, 4:8, :]
                        ohat = SGv[:, :, 8:12, :]
                        shat = SGv[:, :, 12:16, :]

                        t1 = rp[ch].tile([128, 16], BF16, tag="t1")
                        t2 = rp[ch].tile([128, 16], BF16, tag="t2")
                        w_ = rp[ch].tile([128, 16], BF16, tag="w_")
                        cnext = rp[ch].tile([128, 16], BF16, tag="c")
                        th = rp[ch].tile([128, 16], BF16, tag="th")
                        cpv = cprev[ch].rearrange("p (a k s) -> p a k s", a=2, k=4)
                        t1v = t1.rearrange("p (a k s) -> p a k s", a=2, k=4)
                        t2v = t2.rearrange("p (a k s) -> p a k s", a=2, k=4)
                        wv_ = w_.rearrange("p (a k s) -> p a k s", a=2, k=4)
                        cnv = cnext.rearrange("p (a k s) -> p a k s", a=2, k=4)
                        # c = fhat*c_prev + ihat*(2*shat - 1)
                        #   = t1 + 2*(ihat*shat) - ihat
                        nc.vector.tensor_mul(t1v[:], fhat, cpv[:])
                        nc.vector.tensor_mul(t2v[:], ihat, shat)
                        nc.vector.scalar_tensor_tensor(
                            out=wv_[:], in0=t2v[:], scalar=2.0, in1=t1v[:],
                            op0=ALU.mult, op1=ALU.add)
                        nc.vector.tensor_tensor(out=cnv[:], in0=wv_[:],
                                                in1=ihat, op=ALU.subtract)
                        nc.scalar.activation(th[:], cnext[:], AF.Tanh)
                        # h = ohat * tanh(c), both dirs' slot u in one op
                        hout = hv[:, :, u, :, 2 * ch: 2 * ch + 2]
                        thv = th.rearrange("p (a k s) -> p a k s", a=2, k=4)
                        nc.vector.tensor_mul(hout, ohat, thv[:])
                        cprev[ch] = cnext

                    for u in range(S):
                        rstep(0, u)
                        rstep(1, u)

                if lv == 2:
                    probe = pp.tile([1, NS], F32, tag="probe")
                    nc.vector.tensor_copy(probe[:], hsT[0:1, 0:NS])
                    nc.sync.dma_start(out=d_loss[:], in_=probe[:])

            # ---------- emissions ----------
            em_lin = pp.tile([T, N], F32, tag="em_lin")
            exp_em = pp.tile([T, N], F32, tag="exp_em")
            if lv >= 3:
                with tc.tile_pool(name="emc", bufs=1) as ec_, \
                     tc.tile_pool(name="ps_em", bufs=2, space="PSUM") as ps_em:
                    linb = ec_.tile([T, 1], F32, tag="linb")
                    nc.sync.dma_start(out=linb[:], in_=d["linb"][:])
                    hv_e = hsT.rearrange("p (a t k s) -> p a t k s",
                                         a=2, t=S + 1, k=HC)
                    for nb in range(2):
                        toff = nb * 128
                        # fwd: slot == position
                        pe_f = ps_em.tile([T, 512], F32, tag="pef")
                        for k in range(4):
                            rhs = hv_e[:, 0, toff:toff + 128, k:k + 1, :]
                            nc.tensor.matmul(
                                pe_f[:], linT[:, T * k: T * (k + 1)], rhs,
                                start=(k == 0), stop=(k == 3))
                        # bwd: position pos is at slot S-1-pos; slots
                        # [128-toff, 256-toff) cover positions
                        # [toff, toff+128) in DESCENDING order.
                        pe_b = ps_em.tile([T, 512], F32, tag="peb")
                        sb0 = 128 - toff
                        for k in range(4):
                            rhs = hv_e[:, 1, sb0:sb0 + 128, k:k + 1, :]
                            nc.tensor.matmul(
                                pe_b[:], linT[:, T * (4 + k): T * (5 + k)], rhs,
                                start=(k == 0), stop=(k == 3))
                        # em = pe_f + reversed(pe_b) (+ linb); pe_b goes
                        # through SBUF first (DVE TT can read only one PSUM
                        # operand, and the reversed AP stays on SBUF).
                        pb_s = ec_.tile([T, 512], F32, tag="pb_s")
                        nc.vector.tensor_copy(pb_s[:], pe_b[:])
                        pbv = pb_s.rearrange("p (w s) -> p w s", w=128)
                        em_nb = ec_.tile([T, 512], F32, tag="em_nb")
                        env = em_nb.rearrange("p (w s) -> p w s", w=128)
                        pfv = pe_f.rearrange("p (w s) -> p w s", w=128)
                        nc.vector.tensor_tensor(
                            out=env[:], in0=pfv[:], in1=pbv[:, ::-1, :],
                            op=ALU.add)
                        nc.vector.tensor_scalar_add(
                            em_lin[:, 512 * nb: 512 * (nb + 1)], em_nb[:],
                            linb[:])
                        nc.scalar.activation(exp_em[:, 512 * nb: 512 * (nb + 1)],
                                             em_nb[:], AF.Exp, bias=linb[:])
                if lv == 3:
                    probe = pp.tile([1, NS], F32, tag="probe")
                    nc.vector.tensor_copy(probe[:], em_lin[0:1, 0:NS])
                    nc.sync.dma_start(out=d_loss[:], in_=probe[:])

            # ---------- CRF ----------
            if lv >= 4:
                with tc.tile_pool(name="crf", bufs=1) as cp, \
                     tc.tile_pool(name="qs", bufs=3) as qp, \
                     tc.tile_pool(name="ps_q", bufs=2, space="PSUM") as ps_q:
                    trans_sb = cp.tile([T, T], F32, tag="trans")
                    stend = cp.tile([T, 2], F32, tag="stend")
                    estart = cp.tile([T, 1], F32, tag="estart")
                    eend = cp.tile([T, 1], F32, tag="eend")
                    nln45 = cp.tile([T, 1], F32, tag="nln45")
                    ones45 = cp.tile([T, 1], F32, tag="ones45")
                    oh = cp.tile([T, N], F32, tag="oh")
                    oh2 = cp.tile([T, N], F32, tag="oh2")
                    nc.sync.dma_start(out=trans_sb[:], in_=d["trans"][:])
                    nc.sync.dma_start(out=stend[:], in_=d["stend"][:])
                    nc.sync.dma_start(out=oh[:], in_=d["oh"][:])
                    nc.sync.dma_start(out=oh2[:], in_=d["oh2"][:])
                    nc.vector.memset(nln45[:], -LN45)
                    nc.vector.memset(ones45[:], 1.0)
                    nc.scalar.activation(estart[:], stend[:, 0:1], AF.Exp)
                    nc.scalar.activation(eend[:], stend[:, 1:2], AF.Exp)

                    # partition function via two-sided vector chains that
                    # meet at K=127:  Z = sum_i alpha_K(i) * beta_K(i).
                    K = 127
                    Epb = cp.tile([T, T], BF16, tag="Epb")
                    nc.scalar.activation(Epb[:], trans_sb[:], AF.Exp, bias=nln45[:])
                    EpbT = cp.tile([T, T], BF16, tag="EpbT")
                    with tc.tile_pool(name="ps_t", bufs=1, space="PSUM") as ps_t:
                        tpt = ps_t.tile([T, T], BF16, tag="tpt")
                        nc.tensor.transpose(tpt[:], Epb[:], idbf[0:T, 0:T])
                        nc.vector.tensor_copy(EpbT[:], tpt[:])

                    q = qp.tile([T, NS], BF16, tag="q")
                    nc.vector.tensor_scalar_mul(q[:], exp_em[:, 0:NS], estart[:])
                    bq0 = qp.tile([T, NS], BF16, tag="bq")
                    nc.vector.tensor_scalar_mul(
                        bq0[:], eend[:].to_broadcast([T, NS]), ones45[:])
                    bq = bq0                     # beta lives in PSUM after j=1
                    with tc.tile_pool(name="ps_b", bufs=2, space="PSUM") as ps_b:
                        for j in range(1, K + 1):
                            # alpha: t = j
                            sA = ps_q.tile([T, NS], F32, tag="sA")
                            nc.tensor.matmul(sA[:], Epb[:], q[:],
                                             start=True, stop=True)
                            qn = qp.tile([T, NS], BF16, tag="q")
                            nc.vector.tensor_mul(
                                qn[:], sA[:], exp_em[:, NS * j: NS * (j + 1)])
                            q = qn
                            # beta: t = 255 - j
                            t_ = S - 1 - j
                            wv = qp.tile([T, NS], BF16, tag="wv")
                            nc.vector.tensor_mul(
                                wv[:], bq[:],
                                exp_em[:, NS * (t_ + 1): NS * (t_ + 2)])
                            sB = ps_b.tile([T, NS], F32, tag="sB")
                            nc.tensor.matmul(sB[:], EpbT[:], wv[:],
                                             start=True, stop=True)
                            bq = sB
                        # one extra beta step so beta reaches position K
                        wv = qp.tile([T, NS], BF16, tag="wv")
                        nc.vector.tensor_mul(
                            wv[:], bq[:], exp_em[:, NS * (K + 1): NS * (K + 2)])
                        sB = ps_b.tile([T, NS], F32, tag="sB")
                        nc.tensor.matmul(sB[:], EpbT[:], wv[:],
                                         start=True, stop=True)
                        bqf = cp.tile([T, NS], F32, tag="bqf")
                        nc.vector.tensor_copy(bqf[:], sB[:])
                        bq = bqf
                    if lv == 4:
                        probe = pp.tile([1, NS], F32, tag="probe")
                        nc.vector.tensor_copy(probe[:], q[0:1, :])
                        nc.sync.dma_start(out=d_loss[:], in_=probe[:])

                    if lv >= 5:
                        w = cp.tile([T, NS], F32, tag="w")
                        logZ = cp.tile([1, NS], F32, tag="logZ")
                        em_h = cp.tile([1, 2 * NS], F32, tag="em_h")
                        tr_h = cp.tile([1, 2 * NS], F32, tag="tr_h")
                        em_sc = cp.tile([1, NS], F32, tag="em_sc")
                        tr_sc = cp.tile([1, NS], F32, tag="tr_sc")
                        sten_s = cp.tile([1, NS], F32, tag="sten_s")
                        nc.vector.tensor_mul(w[:], q[:], bq[:])
                        with tc.tile_pool(name="ps_f", bufs=1, space="PSUM") as ps_f:
                            sumw = ps_f.tile([1, NS], F32, tag="f1")
                            nc.tensor.matmul(sumw[:], ones45[:], w[:],
                                             start=True, stop=True)
                            nc.scalar.activation(logZ[:], sumw[:], AF.Ln)

                            S1 = cp.tile([T, N], F32, tag="S1")
                            nc.vector.tensor_mul(S1[:], em_lin[:], oh[:])
                            S2 = cp.tile([T, N], F32, tag="S2")
                            for ck in range(2):
                                sl = slice(512 * ck, 512 * (ck + 1))
                                s1p = ps_f.tile([1, 512], F32, tag="fbig")
                                nc.tensor.matmul(s1p[:], ones45[:], S1[:, sl],
                                                 start=True, stop=True)
                                nc.vector.tensor_reduce(
                                    em_h[:, NS * ck: NS * (ck + 1)],
                                    s1p.rearrange("p (t b) -> p b t", b=NS),
                                    axis=mybir.AxisListType.X, op=ALU.add)
                                Rp_ = ps_f.tile([T, 512], F32, tag="fR")
                                nc.tensor.matmul(Rp_[:], trans_sb[:], oh[:, sl],
                                                 start=True, stop=True)
                                nc.vector.tensor_mul(S2[:, sl], Rp_[:], oh2[:, sl])
                                s2p = ps_f.tile([1, 512], F32, tag="fbig2")
                                nc.tensor.matmul(s2p[:], ones45[:], S2[:, sl],
                                                 start=True, stop=True)
                                nc.vector.tensor_reduce(
                                    tr_h[:, NS * ck: NS * (ck + 1)],
                                    s2p.rearrange("p (t b) -> p b t", b=NS),
                                    axis=mybir.AxisListType.X, op=ALU.add)
                            nc.vector.tensor_add(em_sc[:], em_h[:, 0:NS],
                                                 em_h[:, NS:2 * NS])
                            nc.vector.tensor_add(tr_sc[:], tr_h[:, 0:NS],
                                                 tr_h[:, NS:2 * NS])

                            stp = cp.tile([T, NS], F32, tag="stp")
                            enp = cp.tile([T, NS], F32, tag="enp")
                            nc.vector.tensor_scalar_mul(stp[:], oh[:, 0:NS],
                                                        stend[:, 0:1])
                            nc.vector.tensor_scalar_mul(enp[:], oh[:, N - NS:N],
                                                        stend[:, 1:2])
                            sten = ps_f.tile([1, NS], F32, tag="f2")
                            nc.tensor.matmul(sten[:], ones45[:], stp[:],
                                             start=True, stop=False)
                            nc.tensor.matmul(sten[:], ones45[:], enp[:],
                                             start=False, stop=True)
                            nc.vector.tensor_copy(sten_s[:], sten[:])

                        sc1 = cp.tile([1, NS], F32, tag="sc1")
                        sc2 = cp.tile([1, NS], F32, tag="sc2")
                        lossa = cp.tile([1, NS], F32, tag="lossa")
                        lossb = cp.tile([1, NS], F32, tag="lossb")
                        nc.vector.tensor_add(sc1[:], em_sc[:], tr_sc[:])
                        nc.vector.tensor_add(sc2[:], sc1[:], sten_s[:])
                        nc.vector.tensor_tensor(out=lossa[:], in0=logZ[:],
                                                in1=sc2[:], op=ALU.subtract)
                        nc.scalar.activation(lossb[:], lossa[:], AF.Copy,
                                             bias=(S - 1) * LN45)
                        nc.sync.dma_start(out=d_loss[:], in_=lossb[:])

    nc.finalize()
    return nc


def _pack_wT(w, kchunks):
    # w: [M_out rows (gate units, reordered), K] ->
    # [128, (nm*kchunks)*128] tiles: tile (m*kchunks+ec) = w[mU, ecK].T
    M, K = w.shape
    nm = M // 128
    assert K == 128 * kchunks
    tiles = []
    for m in range(nm):
        for ec in range(kchunks):
            blk = w[m * 128:(m + 1) * 128, ec * 128:(ec + 1) * 128]
            tiles.append(np.ascontiguousarray(blk.T))
    return np.concatenate(tiles, axis=1)


def _perm_gates_ifog(w):
    # torch gate order i,f,g,o (blocks of H) -> our chunk order i,f,o,g;
    # g rows scaled by 2 for the sigma-trick (tanh(g) = 2*sigma(2g) - 1)
    i, f, g, o = np.split(w, 4, axis=0)
    return np.concatenate([i, f, o, 2.0 * g], axis=0)


def prepare_in_maps(**inputs):
    x = np.asarray(inputs["x"]).astype(np.int32)          # [32, 256]
    tags = np.asarray(inputs["tags"]).astype(np.int32)
    emb = np.asarray(inputs["emb"], dtype=np.float32)
    lin_w = np.asarray(inputs["lin_w"], dtype=np.float32)
    lin_b = np.asarray(inputs["lin_b"], dtype=np.float32)
    start_t = np.asarray(inputs["start_t"], dtype=np.float32)
    end_t = np.asarray(inputs["end_t"], dtype=np.float32)
    trans = np.asarray(inputs["trans"], dtype=np.float32)

    wihp = {0: _perm_gates_ifog(np.asarray(inputs["w_ih_f"], np.float32)),
            1: _perm_gates_ifog(np.asarray(inputs["w_ih_b"], np.float32))}
    whhp = {0: _perm_gates_ifog(np.asarray(inputs["w_hh_f"], np.float32)),
            1: _perm_gates_ifog(np.asarray(inputs["w_hh_b"], np.float32))}
    bp = {0: _perm_gates_ifog(np.asarray(inputs["b_f"], np.float32).reshape(-1, 1)),
          1: _perm_gates_ifog(np.asarray(inputs["b_b"], np.float32).reshape(-1, 1))}

    wih_t = {dd: _pack_wT(wihp[dd], 2).astype(ml_dtypes.float8_e4m3) for dd in (0, 1)}
    whh_t = {dd: _pack_wT(whhp[dd], 4).astype(ml_dtypes.float8_e4m3) for dd in (0, 1)}

    # biasbc [128, 64]: col = d*32 + m*2 + s' -> b_d[m*128 + p]
    biasbc = np.zeros((128, 64), np.float32)
    for dd in (0, 1):
        for m in range(16):
            col = bp[dd][m * 128:(m + 1) * 128, 0]
            biasbc[:, dd * 32 + 2 * m] = col
            biasbc[:, dd * 32 + 2 * m + 1] = col
    biasbc = biasbc.astype(ml_dtypes.bfloat16)

    # linT [128, 8*T]: tile kc = lin_w[:, kc*128:(kc+1)*128].T (fwd 0-3, bwd 4-7)
    lin_tiles = [np.ascontiguousarray(lin_w[:, kc * 128:(kc + 1) * 128].T)
                 for kc in range(8)]
    linT = np.concatenate(lin_tiles, axis=1).astype(ml_dtypes.bfloat16)

    id128 = np.eye(128, dtype=np.float32)

    in_maps = []
    for core in range(8):
        seqs = slice(4 * core, 4 * core + 4)
        xs = x[seqs]                                      # [4, 256]
        # xidx [128, 8]: col b, row r -> x[s=(r%4), t=(128b+r)//4]
        nflat = xs.T.reshape(-1)                          # n = 4t+s
        xidx = np.ascontiguousarray(nflat.reshape(8, 128).T).astype(np.int32)

        tg = tags[seqs]                                   # [4, 256]
        oh = np.zeros((T, N), np.float32)
        oh[tg.T.reshape(-1), np.arange(N)] = 1.0
        oh2 = np.zeros((T, N), np.float32)
        oh2[:, 0:N - NS] = oh[:, NS:N]

        in_maps.append({
            "emb": emb.astype(ml_dtypes.bfloat16),
            "xidx": xidx,
            "wihf": wih_t[0], "wihb": wih_t[1],
            "whhf": whh_t[0], "whhb": whh_t[1],
            "biasbc": biasbc,
            "linT": linT,
            "linb": lin_b.reshape(T, 1),
            "id128": id128,
            "idbf": np.eye(128, dtype=ml_dtypes.bfloat16),
            "trans": trans,
            "stend": np.stack([start_t, end_t], axis=1),
            "oh": oh,
            "oh2": oh2,
        })
    return in_maps


def get_nc():
    if "nc" not in _cached:
        _cached["nc"] = _build()
    return _cached["nc"]


def kernel(**inputs):
    in_maps = prepare_in_maps(**inputs)
    res = run_bass_kernel_spmd(get_nc(), in_maps, core_ids=list(range(8)))
    total = np.float64(0.0)
    for core in range(8):
        total += np.float64(res.results[core]["loss"]).sum()
    return np.float32(total / 32.0)


# revision 10
# speedup vs baseline: 1.0471x; 1.0471x over previous
"""BiLSTM-CRF NLL loss on 8 Trainium2 NeuronCores.

Sharding: core c owns sequences [4c, 4c+4); each core runs BOTH LSTM
directions and the full CRF for its 4 sequences. No collectives; host sums
8 per-core partials.

Recurrence: TWO phase-shifted chains per core, chain ch owns seqs
{2ch, 2ch+1} and carries BOTH directions in merged instructions. Per
chain-step: one PSUM tile G [128, 64] (cols = d*32 + m*2 + s'), one
accumulation group: bias-inject identity mm (start=True) + 32 x-mms +
64 h-mms (stop on last). sigma-trick: g-gate rows of W/b pre-scaled by 2 so
ONE Sigmoid activation covers all 16 gate chunks (tanh(g) = 2*sigma(2g)-1);
cell update is 4 fused DVE ops (c = t1 + 2*i*s - i), then tanh(c) Act and
the h-mul DVE writes both directions' h slots via a strided AP. This cuts
the per-step serial chain to PE -> Act -> DVE -> Act -> DVE with two
independent chains pipelined to hide the cross-engine latency.

Weights fp8e4m3, x/h matmuls fp8 DoubleRow. h stored fp8 unit-major.

CRF partition function: exp-domain, two-sided (alpha ascends, beta descends,
meet at K=127), bf16 chain operands. Gold-path score via one-hot matmuls.
Loss exits as [1,4] per-core partials.

Self-contained: hardcodes all shapes; only needs numpy + concourse (+ml_dtypes).
"""
import numpy as np
import ml_dtypes

import concourse.bass as bass
import concourse.bacc as bacc
import concourse.tile as tile
from concourse import mybir
from concourse.tile_rust import add_dep_helper
from concourse.bass_utils import run_bass_kernel_spmd

F32 = mybir.dt.float32
FP8 = mybir.dt.float8e4
BF16 = mybir.dt.bfloat16
I32 = mybir.dt.int32
AF = mybir.ActivationFunctionType
ALU = mybir.AluOpType

B, S, E, H, T, V = 32, 256, 256, 512, 45, 50000
NS = 4                 # seqs per core
N = S * NS             # 1024 emission cols, n = 4t+s
NCH = 16               # gate chunks (2048/128)
HC = 4                 # h chunks (512/128)
SW = HC * NS           # state cols per step = 16 (all 4 seqs)
NB_T = 4               # transform n-blocks (of 64 steps = 256 cols each)
TBLK = S // NB_T       # 64 steps per transform block
LN45 = float(np.log(45.0))
DSTRIDE = (S + 1) * SW  # 4112: cols per direction in hsT_all

_cached = {}


def _build(stop_after=None):
    lv = {"xf": 1, "rec": 2, "em": 3, "crf": 4, None: 5}[stop_after]
    nc = bacc.Bacc("TRN2", target_bir_lowering=False, debug=False, num_devices=8)

    d = {}
    d["emb"] = nc.dram_tensor("emb", [V, E], BF16, kind="ExternalInput")
    d["xidx"] = nc.dram_tensor("xidx", [128, 8], I32, kind="ExternalInput")
    d["wihf"] = nc.dram_tensor("wihf", [128, 32 * 128], FP8, kind="ExternalInput")
    d["wihb"] = nc.dram_tensor("wihb", [128, 32 * 128], FP8, kind="ExternalInput")
    d["whhf"] = nc.dram_tensor("whhf", [128, 64 * 128], FP8, kind="ExternalInput")
    d["whhb"] = nc.dram_tensor("whhb", [128, 64 * 128], FP8, kind="ExternalInput")
    d["biasbc"] = nc.dram_tensor("biasbc", [128, 128], BF16, kind="ExternalInput")
    d["linT"] = nc.dram_tensor("linT", [128, 8 * T], BF16, kind="ExternalInput")
    d["linb"] = nc.dram_tensor("linb", [T, 1], F32, kind="ExternalInput")
    d["id128"] = nc.dram_tensor("id128", [128, 128], F32, kind="ExternalInput")
    d["idbf"] = nc.dram_tensor("idbf", [128, 128], BF16, kind="ExternalInput")
    d["trans"] = nc.dram_tensor("trans", [T, T], F32, kind="ExternalInput")
    d["stend"] = nc.dram_tensor("stend", [T, 2], F32, kind="ExternalInput")
    d["oh"] = nc.dram_tensor("oh", [T, N], F32, kind="ExternalInput")
    d["oh2"] = nc.dram_tensor("oh2", [T, N], F32, kind="ExternalInput")
    d_loss = nc.dram_tensor("loss", [1, NS], F32, kind="ExternalOutput")

    with tile.TileContext(nc) as tc:
        with tc.tile_pool(name="persist", bufs=1) as pp, \
             tc.tile_pool(name="gxp", bufs=1) as gxp:
            # persistent weights / tables
            wih = {0: pp.tile([128, 32 * 128], FP8, tag="wihf", name="wihf"),
                   1: pp.tile([128, 32 * 128], FP8, tag="wihb", name="wihb")}
            whh = {0: pp.tile([128, 64 * 128], FP8, tag="whhf", name="whhf"),
                   1: pp.tile([128, 64 * 128], FP8, tag="whhb", name="whhb")}
            biasbc = pp.tile([128, 128], BF16, tag="biasbc")
            ones1 = pp.tile([1, NS], F32, tag="ones1")
            id128 = pp.tile([128, 128], F32, tag="id128")
            idbf = pp.tile([128, 128], BF16, tag="idbf")
            xidx = pp.tile([128, 8], I32, tag="xidx")
            linT = pp.tile([128, 8 * T], BF16, tag="linT")
            nc.sync.dma_start(out=xidx[:], in_=d["xidx"][:])
            nc.sync.dma_start(out=id128[:], in_=d["id128"][:])
            nc.sync.dma_start(out=idbf[:], in_=d["idbf"][:])
            nc.sync.dma_start(out=biasbc[:], in_=d["biasbc"][:])
            nc.vector.memset(ones1[:], 1.0)

            # XT block tiles: [nb] -> [128, 2 ec x 256 n] fp8
            xt = {nb: gxp.tile([128, 2 * TBLK * NS], FP8, tag=f"xt{nb}", name=f"xt{nb}")
                  for nb in range(NB_T)}
            # h state, both dirs in one tile (unit-major):
            # col = d*DSTRIDE + 16*slot + 4*k + s ; slot S = zeros (h0).
            # BOTH directions write slot u at recurrence step u: the bwd
            # direction's h (position S-1-u) is stored TIME-REVERSED, so
            # h-mm reads and the h-write are uniform across dirs. The
            # reversal is undone in the emissions phase via a reversed AP.
            hsT = pp.tile([128, 2 * DSTRIDE], FP8, tag="hsT", name="hsT")
            nc.vector.memset(hsT[:, SW * S: SW * S + SW], 0.0)
            nc.vector.memset(hsT[:, DSTRIDE + SW * S: DSTRIDE + SW * S + SW], 0.0)

            # ---------- phase 0: gather + transpose -> XT ----------
            with tc.tile_pool(name="gat", bufs=3) as gp, \
                 tc.tile_pool(name="ps_tp", bufs=4, space="PSUM") as ps_tp:
                for b in range(8):
                    X = gp.tile([128, E], BF16, tag="X")
                    nc.gpsimd.indirect_dma_start(
                        out=X[:],
                        out_offset=None,
                        in_=d["emb"][:],
                        in_offset=bass.IndirectOffsetOnAxis(ap=xidx[:, b:b + 1], axis=0),
                    )
                    nb, off = b // 2, (b % 2) * 128
                    for ec in range(2):
                        tp = ps_tp.tile([128, 128], BF16, tag="tp")
                        nc.tensor.transpose(tp[:], X[:, 128 * ec: 128 * ec + 128], idbf[:])
                        nc.vector.tensor_copy(
                            xt[nb][:, TBLK * NS * ec + off: TBLK * NS * ec + off + 128],
                            tp[:])

            # weight DMAs after the gathers so they share the DMA engines
            nc.sync.dma_start(out=wih[0][:], in_=d["wihf"][:])
            nc.sync.dma_start(out=wih[1][:], in_=d["wihb"][:])
            nc.sync.dma_start(out=whh[0][:], in_=d["whhf"][:])
            nc.sync.dma_start(out=whh[1][:], in_=d["whhb"][:])
            nc.sync.dma_start(out=linT[:], in_=d["linT"][:])

            # ---------- recurrence ----------
            if lv == 1:
                probe = pp.tile([1, NS], F32, tag="probe")
                nc.vector.tensor_copy(probe[:], xt[0][0:1, 0:NS])
                nc.sync.dma_start(out=d_loss[:], in_=probe[:])
            if lv >= 2:
                with tc.tile_pool(name="rec0", bufs=6) as rp0, \
                     tc.tile_pool(name="rec1", bufs=6) as rp1, \
                     tc.tile_pool(name="psg0", bufs=2, space="PSUM") as pg0, \
                     tc.tile_pool(name="psg1", bufs=2, space="PSUM") as pg1:
                    rp = [rp0, rp1]
                    pg = [pg0, pg1]
                    cprev = [None, None]
                    for ch in (0, 1):
                        cinit = rp[ch].tile([128, 16], BF16, tag="c")
                        nc.vector.memset(cinit[:], 0.0)
                        cprev[ch] = cinit

                    DR = mybir.MatmulPerfMode.DoubleRow
                    xtv = {nb: xt[nb].rearrange("p (e c) -> p e c", e=2)
                           for nb in range(NB_T)}
                    hv = hsT.rearrange("p (a t k s) -> p a t k s",
                                       a=2, t=S + 1, k=HC, s=NS)
                    offset_anchor = [None]   # set to chain0's first tanh_c

                    def rstep(dd, u):
                        # chain dd = ONE direction, all 4 seqs. Both dirs
                        # write slot u, read slot u-1 (u=0: slot S = zeros);
                        # bwd h is stored time-reversed.
                        slot_r = S if u == 0 else u - 1
                        col = u if dd == 0 else S - 1 - u   # xt timestep

                        G = pg[dd].tile([128, 64], F32, tag="G")
                        # one accumulation group per G tile: bias-inject
                        # (start=True zeroes the region) -> x-mms -> h-mms
                        # (stop=True on the very last; at u=0 there are no
                        # h-mms so the last x-mm stops). add_dep_helper pins
                        # start-first / stop-last against scheduler
                        # reordering. Phase A (inject + x) has no h dep and
                        # runs in the previous step's tail.
                        mms = []
                        mm = nc.tensor.matmul(
                            G[:], idbf[:],
                            biasbc[:, dd * 64: dd * 64 + 64],
                            start=True, stop=False)
                        mms.append(mm)
                        nb = col // TBLK
                        j = col % TBLK
                        for m in range(NCH):
                            sl = G[:, 4 * m: 4 * m + 4]
                            wpair = wih[dd][:, 2 * m * 128:(2 * m + 2) * 128]
                            mm = nc.tensor.matmul(
                                sl,
                                wpair.rearrange("p (c f) -> p c f", c=2),
                                xtv[nb][:, :, NS * j: NS * j + NS],
                                start=False,
                                stop=(u == 0 and m == NCH - 1),
                                perf_mode=DR)
                            mms.append(mm)
                        if u > 0:
                            for m in range(NCH):
                                sl = G[:, 4 * m: 4 * m + 4]
                                for kp in range(HC // 2):
                                    hpair = whh[dd][
                                        :, (4 * m + 2 * kp) * 128:
                                        (4 * m + 2 * kp + 2) * 128]
                                    rhs = hv[:, dd, slot_r,
                                             2 * kp: 2 * kp + 2, :]
                                    mm = nc.tensor.matmul(
                                        sl,
                                        hpair.rearrange(
                                            "p (c f) -> p c f", c=2),
                                        rhs,
                                        start=False,
                                        stop=(m == NCH - 1
                                              and kp == HC // 2 - 1),
                                        perf_mode=DR)
                                    mms.append(mm)
                        first, last = mms[0], mms[-1]
                        for mm in mms[1:]:
                            add_dep_helper(mm.ins, first.ins, sync=False,
                                           reason="group start first")
                        for mm in mms[:-1]:
                            add_dep_helper(last.ins, mm.ins, sync=False,
                                           reason="group stop last")

                        # sigma over ALL 64 gate cols (g rows pre-scaled x2)
                        SG = rp[dd].tile([128, 64], BF16, tag="SG")
                        act_sg = nc.scalar.activation(SG[:], G[:], AF.Sigmoid)
                        if u == 0 and dd == 1 and offset_anchor[0] is not None:
                            # seed a half-loop phase offset between the two
                            # chains so they pipeline instead of locksteping
                            add_dep_helper(act_sg.ins, offset_anchor[0].ins,
                                           reason="phase offset seed")
                        SGv = SG.rearrange("p (m s) -> p m s", m=NCH)
                        ihat = SGv[:, 0:4, :]
                        fhat = SGv[:, 4:8, :]
                        ohat = SGv[:, 8:12, :]
                        shat = SGv[:, 12:16, :]

                        t1 = rp[dd].tile([128, 16], BF16, tag="t1")
                        t2 = rp[dd].tile([128, 16], BF16, tag="t2")
                        w_ = rp[dd].tile([128, 16], BF16, tag="w_")
                        cnext = rp[dd].tile([128, 16], BF16, tag="c")
                        th = rp[dd].tile([128, 16], BF16, tag="th")
                        cpv = cprev[dd].rearrange("p (k s) -> p k s", k=4)
                        t1v = t1.rearrange("p (k s) -> p k s", k=4)
                        t2v = t2.rearrange("p (k s) -> p k s", k=4)
                        wv_ = w_.rearrange("p (k s) -> p k s", k=4)
                        cnv = cnext.rearrange("p (k s) -> p k s", k=4)
                        # c = fhat*c_prev + ihat*(2*shat - 1)
                        #   = t1 + 2*(ihat*shat) - ihat
                        nc.vector.tensor_mul(t1v[:], fhat, cpv[:])
                        nc.vector.tensor_mul(t2v[:], ihat, shat)
                        nc.vector.scalar_tensor_tensor(
                            out=wv_[:], in0=t2v[:], scalar=2.0, in1=t1v[:],
                            op0=ALU.mult, op1=ALU.add)
                        nc.vector.tensor_tensor(out=cnv[:], in0=wv_[:],
                                                in1=ihat, op=ALU.subtract)
                        act_th = nc.scalar.activation(th[:], cnext[:], AF.Tanh)
                        if u == 0 and dd == 0:
                            offset_anchor[0] = act_th
                        # h = ohat * tanh(c) -> this dir's slot u
                        hout = hv[:, dd, u, :, :]
                        thv = th.rearrange("p (k s) -> p k s", k=4)
                        nc.vector.tensor_mul(hout, ohat, thv[:])
                        cprev[dd] = cnext

                    for u in range(S):
                        rstep(0, u)
                        rstep(1, u)

                if lv == 2:
                    probe = pp.tile([1, NS], F32, tag="probe")
                    nc.vector.tensor_copy(probe[:], hsT[0:1, 0:NS])
                    nc.sync.dma_start(out=d_loss[:], in_=probe[:])

            # ---------- emissions ----------
            em_lin = pp.tile([T, N], F32, tag="em_lin")
            exp_em = pp.tile([T, N], F32, tag="exp_em")
            if lv >= 3:
                with tc.tile_pool(name="emc", bufs=1) as ec_, \
                     tc.tile_pool(name="ps_em", bufs=2, space="PSUM") as ps_em:
                    linb = ec_.tile([T, 1], F32, tag="linb")
                    nc.sync.dma_start(out=linb[:], in_=d["linb"][:])
                    hv_e = hsT.rearrange("p (a t k s) -> p a t k s",
                                         a=2, t=S + 1, k=HC)
                    for nb in range(2):
                        toff = nb * 128
                        # fwd: slot == position
                        pe_f = ps_em.tile([T, 512], F32, tag="pef")
                        for k in range(4):
                            rhs = hv_e[:, 0, toff:toff + 128, k:k + 1, :]
                            nc.tensor.matmul(
                                pe_f[:], linT[:, T * k: T * (k + 1)], rhs,
                                start=(k == 0), stop=(k == 3))
                        # bwd: position pos is at slot S-1-pos; slots
                        # [128-toff, 256-toff) cover positions
                        # [toff, toff+128) in DESCENDING order.
                        pe_b = ps_em.tile([T, 512], F32, tag="peb")
                        sb0 = 128 - toff
                        for k in range(4):
                            rhs = hv_e[:, 1, sb0:sb0 + 128, k:k + 1, :]
                            nc.tensor.matmul(
                                pe_b[:], linT[:, T * (4 + k): T * (5 + k)], rhs,
                                start=(k == 0), stop=(k == 3))
                        # em = pe_f + reversed(pe_b) (+ linb); pe_b goes
                        # through SBUF first (DVE TT can read only one PSUM
                        # operand, and the reversed AP stays on SBUF).
                        pb_s = ec_.tile([T, 512], F32, tag="pb_s")
                        nc.vector.tensor_copy(pb_s[:], pe_b[:])
                        pbv = pb_s.rearrange("p (w s) -> p w s", w=128)
                        em_nb = ec_.tile([T, 512], F32, tag="em_nb")
                        env = em_nb.rearrange("p (w s) -> p w s", w=128)
                        pfv = pe_f.rearrange("p (w s) -> p w s", w=128)
                        nc.vector.tensor_tensor(
                            out=env[:], in0=pfv[:], in1=pbv[:, ::-1, :],
                            op=ALU.add)
                        nc.vector.tensor_scalar_add(
                            em_lin[:, 512 * nb: 512 * (nb + 1)], em_nb[:],
                            linb[:])
                        nc.scalar.activation(exp_em[:, 512 * nb: 512 * (nb + 1)],
                                             em_nb[:], AF.Exp, bias=linb[:])
                if lv == 3:
                    probe = pp.tile([1, NS], F32, tag="probe")
                    nc.vector.tensor_copy(probe[:], em_lin[0:1, 0:NS])
                    nc.sync.dma_start(out=d_loss[:], in_=probe[:])

            # ---------- CRF ----------
            if lv >= 4:
                with tc.tile_pool(name="crf", bufs=1) as cp, \
                     tc.tile_pool(name="qs", bufs=3) as qp, \
                     tc.tile_pool(name="ps_q", bufs=2, space="PSUM") as ps_q:
                    trans_sb = cp.tile([T, T], F32, tag="trans")
                    stend = cp.tile([T, 2], F32, tag="stend")
                    estart = cp.tile([T, 1], F32, tag="estart")
                    eend = cp.tile([T, 1], F32, tag="eend")
                    nln45 = cp.tile([T, 1], F32, tag="nln45")
                    ones45 = cp.tile([T, 1], F32, tag="ones45")
                    oh = cp.tile([T, N], F32, tag="oh")
                    oh2 = cp.tile([T, N], F32, tag="oh2")
                    nc.sync.dma_start(out=trans_sb[:], in_=d["trans"][:])
                    nc.sync.dma_start(out=stend[:], in_=d["stend"][:])
                    nc.sync.dma_start(out=oh[:], in_=d["oh"][:])
                    nc.sync.dma_start(out=oh2[:], in_=d["oh2"][:])
                    nc.vector.memset(nln45[:], -LN45)
                    nc.vector.memset(ones45[:], 1.0)
                    nc.scalar.activation(estart[:], stend[:, 0:1], AF.Exp)
                    nc.scalar.activation(eend[:], stend[:, 1:2], AF.Exp)

                    # partition function via two-sided vector chains that
                    # meet at K=127:  Z = sum_i alpha_K(i) * beta_K(i).
                    K = 127
                    Epb = cp.tile([T, T], BF16, tag="Epb")
                    nc.scalar.activation(Epb[:], trans_sb[:], AF.Exp, bias=nln45[:])
                    EpbT = cp.tile([T, T], BF16, tag="EpbT")
                    with tc.tile_pool(name="ps_t", bufs=1, space="PSUM") as ps_t:
                        tpt = ps_t.tile([T, T], BF16, tag="tpt")
                        nc.tensor.transpose(tpt[:], Epb[:], idbf[0:T, 0:T])
                        nc.vector.tensor_copy(EpbT[:], tpt[:])

                    q = qp.tile([T, NS], BF16, tag="q")
                    nc.vector.tensor_scalar_mul(q[:], exp_em[:, 0:NS], estart[:])
                    bq0 = qp.tile([T, NS], BF16, tag="bq")
                    nc.vector.tensor_scalar_mul(
                        bq0[:], eend[:].to_broadcast([T, NS]), ones45[:])
                    bq = bq0                     # beta lives in PSUM after j=1
                    with tc.tile_pool(name="ps_b", bufs=2, space="PSUM") as ps_b:
                        for j in range(1, K + 1):
                            # alpha: t = j
                            sA = ps_q.tile([T, NS], F32, tag="sA")
                            nc.tensor.matmul(sA[:], Epb[:], q[:],
                                             start=True, stop=True)
                            qn = qp.tile([T, NS], BF16, tag="q")
                            nc.vector.tensor_mul(
                                qn[:], sA[:], exp_em[:, NS * j: NS * (j + 1)])
                            q = qn
                            # beta: t = 255 - j
                            t_ = S - 1 - j
                            wv = qp.tile([T, NS], BF16, tag="wv")
                            nc.vector.tensor_mul(
                                wv[:], bq[:],
                                exp_em[:, NS * (t_ + 1): NS * (t_ + 2)])
                            sB = ps_b.tile([T, NS], F32, tag="sB")
                            nc.tensor.matmul(sB[:], EpbT[:], wv[:],
                                             start=True, stop=True)
                            bq = sB
                        # one extra beta step so beta reaches position K
                        wv = qp.tile([T, NS], BF16, tag="wv")
                        nc.vector.tensor_mul(
                            wv[:], bq[:], exp_em[:, NS * (K + 1): NS * (K + 2)])
                        sB = ps_b.tile([T, NS], F32, tag="sB")
                        nc.tensor.matmul(sB[:], EpbT[:], wv[:],
                                         start=True, stop=True)
                        bqf = cp.tile([T, NS], F32, tag="bqf")
                        nc.vector.tensor_copy(bqf[:], sB[:])
                        bq = bqf
                    if lv == 4:
                        probe = pp.tile([1, NS], F32, tag="probe")
                        nc.vector.tensor_copy(probe[:], q[0:1, :])
                        nc.sync.dma_start(out=d_loss[:], in_=probe[:])

                    if lv >= 5:
                        w = cp.tile([T, NS], F32, tag="w")
                        logZ = cp.tile([1, NS], F32, tag="logZ")
                        em_h = cp.tile([1, 2 * NS], F32, tag="em_h")
                        tr_h = cp.tile([1, 2 * NS], F32, tag="tr_h")
                        em_sc = cp.tile([1, NS], F32, tag="em_sc")
                        tr_sc = cp.tile([1, NS], F32, tag="tr_sc")
                        sten_s = cp.tile([1, NS], F32, tag="sten_s")
                        nc.vector.tensor_mul(w[:], q[:], bq[:])
                        with tc.tile_pool(name="ps_f", bufs=1, space="PSUM") as ps_f:
                            sumw = ps_f.tile([1, NS], F32, tag="f1")
                            nc.tensor.matmul(sumw[:], ones45[:], w[:],
                                             start=True, stop=True)
                            nc.scalar.activation(logZ[:], sumw[:], AF.Ln)

                            S1 = cp.tile([T, N], F32, tag="S1")
                            nc.vector.tensor_mul(S1[:], em_lin[:], oh[:])
                            S2 = cp.tile([T, N], F32, tag="S2")
                            for ck in range(2):
                                sl = slice(512 * ck, 512 * (ck + 1))
                                s1p = ps_f.tile([1, 512], F32, tag="fbig")
                                nc.tensor.matmul(s1p[:], ones45[:], S1[:, sl],
                                                 start=True, stop=True)
                                nc.vector.tensor_reduce(
                                    em_h[:, NS * ck: NS * (ck + 1)],
                                    s1p.rearrange("p (t b) -> p b t", b=NS),
                                    axis=mybir.AxisListType.X, op=ALU.add)
                                Rp_ = ps_f.tile([T, 512], F32, tag="fR")
                                nc.tensor.matmul(Rp_[:], trans_sb[:], oh[:, sl],
                                                 start=True, stop=True)
                                nc.vector.tensor_mul(S2[:, sl], Rp_[:], oh2[:, sl])
                                s2p = ps_f.tile([1, 512], F32, tag="fbig2")
                                nc.tensor.matmul(s2p[:], ones45[:], S2[:, sl],
                                                 start=True, stop=True)
                                nc.vector.tensor_reduce(
                                    tr_h[:, NS * ck: NS * (ck + 1)],
                                    s2p.rearrange("p (t b) -> p b t", b=NS),
                                    axis=mybir.AxisListType.X, op=ALU.add)
                            nc.vector.tensor_add(em_sc[:], em_h[:, 0:NS],
                                                 em_h[:, NS:2 * NS])
                            nc.vector.tensor_add(tr_sc[:], tr_h[:, 0:NS],
                                                 tr_h[:, NS:2 * NS])

                            stp = cp.tile([T, NS], F32, tag="stp")
                            enp = cp.tile([T, NS], F32, tag="enp")
                            nc.vector.tensor_scalar_mul(stp[:], oh[:, 0:NS],
                                                        stend[:, 0:1])
                            nc.vector.tensor_scalar_mul(enp[:], oh[:, N - NS:N],
                                                        stend[:, 1:2])
                            sten = ps_f.tile([1, NS], F32, tag="f2")
                            nc.tensor.matmul(sten[:], ones45[:], stp[:],
                                             start=True, stop=False)
                            nc.tensor.matmul(sten[:], ones45[:], enp[:],
                                             start=False, stop=True)
                            nc.vector.tensor_copy(sten_s[:], sten[:])

                        sc1 = cp.tile([1, NS], F32, tag="sc1")
                        sc2 = cp.tile([1, NS], F32, tag="sc2")
                        lossa = cp.tile([1, NS], F32, tag="lossa")
                        lossb = cp.tile([1, NS], F32, tag="lossb")
                        nc.vector.tensor_add(sc1[:], em_sc[:], tr_sc[:])
                        nc.vector.tensor_add(sc2[:], sc1[:], sten_s[:])
                        nc.vector.tensor_tensor(out=lossa[:], in0=logZ[:],
                                                in1=sc2[:], op=ALU.subtract)
                        nc.scalar.activation(lossb[:], lossa[:], AF.Copy,
                                             bias=(S - 1) * LN45)
                        nc.sync.dma_start(out=d_loss[:], in_=lossb[:])

    nc.finalize()
    return nc


def _pack_wT(w, kchunks):
    # w: [M_out rows (gate units, reordered), K] ->
    # [128, (nm*kchunks)*128] tiles: tile (m*kchunks+ec) = w[mU, ecK].T
    M, K = w.shape
    nm = M // 128
    assert K == 128 * kchunks
    tiles = []
    for m in range(nm):
        for ec in range(kchunks):
            blk = w[m * 128:(m + 1) * 128, ec * 128:(ec + 1) * 128]
            tiles.append(np.ascontiguousarray(blk.T))
    return np.concatenate(tiles, axis=1)


def _perm_gates_ifog(w):
    # torch gate order i,f,g,o (blocks of H) -> our chunk order i,f,o,g;
    # g rows scaled by 2 for the sigma-trick (tanh(g) = 2*sigma(2g) - 1)
    i, f, g, o = np.split(w, 4, axis=0)
    return np.concatenate([i, f, o, 2.0 * g], axis=0)


def prepare_in_maps(**inputs):
    x = np.asarray(inputs["x"]).astype(np.int32)          # [32, 256]
    tags = np.asarray(inputs["tags"]).astype(np.int32)
    emb = np.asarray(inputs["emb"], dtype=np.float32)
    lin_w = np.asarray(inputs["lin_w"], dtype=np.float32)
    lin_b = np.asarray(inputs["lin_b"], dtype=np.float32)
    start_t = np.asarray(inputs["start_t"], dtype=np.float32)
    end_t = np.asarray(inputs["end_t"], dtype=np.float32)
    trans = np.asarray(inputs["trans"], dtype=np.float32)

    wihp = {0: _perm_gates_ifog(np.asarray(inputs["w_ih_f"], np.float32)),
            1: _perm_gates_ifog(np.asarray(inputs["w_ih_b"], np.float32))}
    whhp = {0: _perm_gates_ifog(np.asarray(inputs["w_hh_f"], np.float32)),
            1: _perm_gates_ifog(np.asarray(inputs["w_hh_b"], np.float32))}
    bp = {0: _perm_gates_ifog(np.asarray(inputs["b_f"], np.float32).reshape(-1, 1)),
          1: _perm_gates_ifog(np.asarray(inputs["b_b"], np.float32).reshape(-1, 1))}

    wih_t = {dd: _pack_wT(wihp[dd], 2).astype(ml_dtypes.float8_e4m3) for dd in (0, 1)}
    whh_t = {dd: _pack_wT(whhp[dd], 4).astype(ml_dtypes.float8_e4m3) for dd in (0, 1)}

    # biasbc [128, 128]: col = d*64 + m*4 + s -> b_d[m*128 + p]
    biasbc = np.zeros((128, 128), np.float32)
    for dd in (0, 1):
        for m in range(16):
            col = bp[dd][m * 128:(m + 1) * 128, 0]
            for s in range(4):
                biasbc[:, dd * 64 + 4 * m + s] = col
    biasbc = biasbc.astype(ml_dtypes.bfloat16)

    # linT [128, 8*T]: tile kc = lin_w[:, kc*128:(kc+1)*128].T (fwd 0-3, bwd 4-7)
    lin_tiles = [np.ascontiguousarray(lin_w[:, kc * 128:(kc + 1) * 128].T)
                 for kc in range(8)]
    linT = np.concatenate(lin_tiles, axis=1).astype(ml_dtypes.bfloat16)

    id128 = np.eye(128, dtype=np.float32)

    in_maps = []
    for core in range(8):
        seqs = slice(4 * core, 4 * core + 4)
        xs = x[seqs]                                      # [4, 256]
        # xidx [128, 8]: col b, row r -> x[s=(r%4), t=(128b+r)//4]
        nflat = xs.T.reshape(-1)                          # n = 4t+s
        xidx = np.ascontiguousarray(nflat.reshape(8, 128).T).astype(np.int32)

        tg = tags[seqs]                                   # [4, 256]
        oh = np.zeros((T, N), np.float32)
        oh[tg.T.reshape(-1), np.arange(N)] = 1.0
        oh2 = np.zeros((T, N), np.float32)
        oh2[:, 0:N - NS] = oh[:, NS:N]

        in_maps.append({
            "emb": emb.astype(ml_dtypes.bfloat16),
            "xidx": xidx,
            "wihf": wih_t[0], "wihb": wih_t[1],
            "whhf": whh_t[0], "whhb": whh_t[1],
            "biasbc": biasbc,
            "linT": linT,
            "linb": lin_b.reshape(T, 1),
            "id128": id128,
            "idbf": np.eye(128, dtype=ml_dtypes.bfloat16),
            "trans": trans,
            "stend": np.stack([start_t, end_t], axis=1),
            "oh": oh,
            "oh2": oh2,
        })
    return in_maps


def get_nc():
    if "nc" not in _cached:
        _cached["nc"] = _build()
    return _cached["nc"]


def kernel(**inputs):
    in_maps = prepare_in_maps(**inputs)
    res = run_bass_kernel_spmd(get_nc(), in_maps, core_ids=list(range(8)))
    total = np.float64(0.0)
    for core in range(8):
        total += np.float64(res.results[core]["loss"]).sum()
    return np.float32(total / 32.0)


# revision 12
# speedup vs baseline: 1.1362x; 1.0851x over previous
"""BiLSTM-CRF NLL loss on 8 Trainium2 NeuronCores.

Sharding: core c owns sequences [4c, 4c+4); each core runs BOTH LSTM
directions and the full CRF for its 4 sequences. No collectives; host sums
8 per-core partials.

Recurrence: TWO phase-shifted chains per core, chain ch owns seqs
{2ch, 2ch+1} and carries BOTH directions in merged instructions. Per
chain-step: one PSUM tile G [128, 64] (cols = d*32 + m*2 + s'), one
accumulation group: bias-inject identity mm (start=True) + 32 x-mms +
64 h-mms (stop on last). sigma-trick: g-gate rows of W/b pre-scaled by 2 so
ONE Sigmoid activation covers all 16 gate chunks (tanh(g) = 2*sigma(2g)-1);
cell update is 4 fused DVE ops (c = t1 + 2*i*s - i), then tanh(c) Act and
the h-mul DVE writes both directions' h slots via a strided AP. This cuts
the per-step serial chain to PE -> Act -> DVE -> Act -> DVE with two
independent chains pipelined to hide the cross-engine latency.

Weights fp8e4m3, x/h matmuls fp8 DoubleRow. h stored fp8 unit-major.

CRF partition function: exp-domain, two-sided (alpha ascends, beta descends,
meet at K=127), bf16 chain operands. Gold-path score via one-hot matmuls.
Loss exits as [1,4] per-core partials.

Self-contained: hardcodes all shapes; only needs numpy + concourse (+ml_dtypes).
"""
import numpy as np
import ml_dtypes

import concourse.bass as bass
import concourse.bacc as bacc
import concourse.tile as tile
from concourse import mybir
from concourse.tile_rust import add_dep_helper
from concourse.bass_utils import run_bass_kernel_spmd

F32 = mybir.dt.float32
FP8 = mybir.dt.float8e4
BF16 = mybir.dt.bfloat16
I32 = mybir.dt.int32
AF = mybir.ActivationFunctionType
ALU = mybir.AluOpType

B, S, E, H, T, V = 32, 256, 256, 512, 45, 50000
NS = 4                 # seqs per core
N = S * NS             # 1024 emission cols, n = 4t+s
NCH = 16               # gate chunks (2048/128)
HC = 4                 # h chunks (512/128)
SW = HC * NS           # state cols per step = 16 (all 4 seqs)
NB_T = 4               # transform n-blocks (of 64 steps = 256 cols each)
TBLK = S // NB_T       # 64 steps per transform block
LN45 = float(np.log(45.0))
DSTRIDE = (S + 1) * SW  # 4112: cols per direction in hsT_all

_cached = {}


def _build(stop_after=None):
    lv = {"xf": 1, "rec": 2, "em": 3, "crf": 4, None: 5}[stop_after]
    nc = bacc.Bacc("TRN2", target_bir_lowering=False, debug=False, num_devices=8)

    d = {}
    d["emb"] = nc.dram_tensor("emb", [V, E], BF16, kind="ExternalInput")
    d["xidx"] = nc.dram_tensor("xidx", [128, 8], I32, kind="ExternalInput")
    d["wihf"] = nc.dram_tensor("wihf", [128, 32 * 128], FP8, kind="ExternalInput")
    d["wihb"] = nc.dram_tensor("wihb", [128, 32 * 128], FP8, kind="ExternalInput")
    d["whhf"] = nc.dram_tensor("whhf", [128, 64 * 128], FP8, kind="ExternalInput")
    d["whhb"] = nc.dram_tensor("whhb", [128, 64 * 128], FP8, kind="ExternalInput")
    d["biasbc"] = nc.dram_tensor("biasbc", [128, 128], BF16, kind="ExternalInput")
    d["linT"] = nc.dram_tensor("linT", [128, 8 * T], BF16, kind="ExternalInput")
    d["linb"] = nc.dram_tensor("linb", [T, 1], F32, kind="ExternalInput")
    d["id128"] = nc.dram_tensor("id128", [128, 128], F32, kind="ExternalInput")
    d["idbf"] = nc.dram_tensor("idbf", [128, 128], BF16, kind="ExternalInput")
    d["trans"] = nc.dram_tensor("trans", [T, T], F32, kind="ExternalInput")
    d["stend"] = nc.dram_tensor("stend", [T, 2], F32, kind="ExternalInput")
    d["oh"] = nc.dram_tensor("oh", [T, N], F32, kind="ExternalInput")
    d["oh2"] = nc.dram_tensor("oh2", [T, N], F32, kind="ExternalInput")
    d_loss = nc.dram_tensor("loss", [1, NS], F32, kind="ExternalOutput")

    with tile.TileContext(nc) as tc:
        with tc.tile_pool(name="persist", bufs=1) as pp, \
             tc.tile_pool(name="gxp", bufs=1) as gxp:
            # persistent weights / tables
            wih = {0: pp.tile([128, 32 * 128], FP8, tag="wihf", name="wihf"),
                   1: pp.tile([128, 32 * 128], FP8, tag="wihb", name="wihb")}
            whh = {0: pp.tile([128, 64 * 128], FP8, tag="whhf", name="whhf"),
                   1: pp.tile([128, 64 * 128], FP8, tag="whhb", name="whhb")}
            biasbc = pp.tile([128, 128], BF16, tag="biasbc")
            ones1 = pp.tile([1, NS], F32, tag="ones1")
            id128 = pp.tile([128, 128], F32, tag="id128")
            idbf = pp.tile([128, 128], BF16, tag="idbf")
            xidx = pp.tile([128, 8], I32, tag="xidx")
            linT = pp.tile([128, 8 * T], BF16, tag="linT")
            nc.sync.dma_start(out=xidx[:], in_=d["xidx"][:])
            nc.sync.dma_start(out=id128[:], in_=d["id128"][:])
            nc.sync.dma_start(out=idbf[:], in_=d["idbf"][:])
            nc.sync.dma_start(out=biasbc[:], in_=d["biasbc"][:])
            nc.vector.memset(ones1[:], 1.0)

            # XT block tiles: [nb] -> [128, 2 ec x 256 n] fp8
            xt = {nb: gxp.tile([128, 2 * TBLK * NS], FP8, tag=f"xt{nb}", name=f"xt{nb}")
                  for nb in range(NB_T)}
            # h state, both dirs in one tile (unit-major):
            # col = d*DSTRIDE + 16*slot + 4*k + s ; slot S = zeros (h0).
            # BOTH directions write slot u at recurrence step u: the bwd
            # direction's h (position S-1-u) is stored TIME-REVERSED, so
            # h-mm reads and the h-write are uniform across dirs. The
            # reversal is undone in the emissions phase via a reversed AP.
            hsT = pp.tile([128, 2 * DSTRIDE], FP8, tag="hsT", name="hsT")
            nc.vector.memset(hsT[:, SW * S: SW * S + SW], 0.0)
            nc.vector.memset(hsT[:, DSTRIDE + SW * S: DSTRIDE + SW * S + SW], 0.0)

            # ---------- phase 0: gather + transpose -> XT ----------
            with tc.tile_pool(name="gat", bufs=3) as gp, \
                 tc.tile_pool(name="ps_tp", bufs=4, space="PSUM") as ps_tp:
                for b in range(8):
                    X = gp.tile([128, E], BF16, tag="X")
                    nc.gpsimd.indirect_dma_start(
                        out=X[:],
                        out_offset=None,
                        in_=d["emb"][:],
                        in_offset=bass.IndirectOffsetOnAxis(ap=xidx[:, b:b + 1], axis=0),
                    )
                    nb, off = b // 2, (b % 2) * 128
                    for ec in range(2):
                        tp = ps_tp.tile([128, 128], BF16, tag="tp")
                        nc.tensor.transpose(tp[:], X[:, 128 * ec: 128 * ec + 128], idbf[:])
                        nc.vector.tensor_copy(
                            xt[nb][:, TBLK * NS * ec + off: TBLK * NS * ec + off + 128],
                            tp[:])

            # weight DMAs after the gathers so they share the DMA engines
            nc.sync.dma_start(out=wih[0][:], in_=d["wihf"][:])
            nc.sync.dma_start(out=wih[1][:], in_=d["wihb"][:])
            nc.sync.dma_start(out=whh[0][:], in_=d["whhf"][:])
            nc.sync.dma_start(out=whh[1][:], in_=d["whhb"][:])
            nc.sync.dma_start(out=linT[:], in_=d["linT"][:])

            # ---------- recurrence ----------
            if lv == 1:
                probe = pp.tile([1, NS], F32, tag="probe")
                nc.vector.tensor_copy(probe[:], xt[0][0:1, 0:NS])
                nc.sync.dma_start(out=d_loss[:], in_=probe[:])
            if lv >= 2:
                with tc.tile_pool(name="rec0", bufs=6) as rp0, \
                     tc.tile_pool(name="rec1", bufs=6) as rp1, \
                     tc.tile_pool(name="psg0", bufs=2, space="PSUM") as pg0, \
                     tc.tile_pool(name="psg1", bufs=2, space="PSUM") as pg1:
                    rp = [rp0, rp1]
                    pg = [pg0, pg1]
                    cprev = [None, None]
                    for ch in (0, 1):
                        cinit = rp[ch].tile([128, 16], BF16, tag="c")
                        nc.vector.memset(cinit[:], 0.0)
                        cprev[ch] = cinit

                    DR = mybir.MatmulPerfMode.DoubleRow
                    xtv = {nb: xt[nb].rearrange("p (e c) -> p e c", e=2)
                           for nb in range(NB_T)}
                    hv = hsT.rearrange("p (a t k s) -> p a t k s",
                                       a=2, t=S + 1, k=HC, s=NS)
                    def stage_pe(dd, u):
                        # chain dd = ONE direction, all 4 seqs. Both dirs
                        # write slot u, read slot u-1 (u=0: slot S = zeros);
                        # bwd h is stored time-reversed.
                        slot_r = S if u == 0 else u - 1
                        col = u if dd == 0 else S - 1 - u   # xt timestep

                        G = pg[dd].tile([128, 64], F32, tag="G")
                        # one accumulation group per G tile: bias-inject
                        # (start=True zeroes the region) -> x-mms -> h-mms
                        # (stop=True on the very last; at u=0 there are no
                        # h-mms so the last x-mm stops). add_dep_helper pins
                        # start-first / stop-last against scheduler
                        # reordering. Phase A (inject + x) has no h dep and
                        # runs in the previous step's tail.
                        mms = []
                        mm = nc.tensor.matmul(
                            G[:], idbf[:],
                            biasbc[:, dd * 64: dd * 64 + 64],
                            start=True, stop=False)
                        mms.append(mm)
                        nb = col // TBLK
                        j = col % TBLK
                        for m in range(NCH):
                            sl = G[:, 4 * m: 4 * m + 4]
                            wpair = wih[dd][:, 2 * m * 128:(2 * m + 2) * 128]
                            mm = nc.tensor.matmul(
                                sl,
                                wpair.rearrange("p (c f) -> p c f", c=2),
                                xtv[nb][:, :, NS * j: NS * j + NS],
                                start=False,
                                stop=(u == 0 and m == NCH - 1),
                                perf_mode=DR)
                            mms.append(mm)
                        if u > 0:
                            for m in range(NCH):
                                sl = G[:, 4 * m: 4 * m + 4]
                                for kp in range(HC // 2):
                                    hpair = whh[dd][
                                        :, (4 * m + 2 * kp) * 128:
                                        (4 * m + 2 * kp + 2) * 128]
                                    rhs = hv[:, dd, slot_r,
                                             2 * kp: 2 * kp + 2, :]
                                    mm = nc.tensor.matmul(
                                        sl,
                                        hpair.rearrange(
                                            "p (c f) -> p c f", c=2),
                                        rhs,
                                        start=False,
                                        stop=(m == NCH - 1
                                              and kp == HC // 2 - 1),
                                        perf_mode=DR)
                                    mms.append(mm)
                        first, last = mms[0], mms[-1]
                        for mm in mms[1:]:
                            add_dep_helper(mm.ins, first.ins, sync=False,
                                           reason="group start first")
                        for mm in mms[:-1]:
                            add_dep_helper(last.ins, mm.ins, sync=False,
                                           reason="group stop last")
                        return G

                    def stage_sig(dd, G):
                        # sigma over ALL 64 gate cols (g rows pre-scaled x2)
                        SG = rp[dd].tile([128, 64], BF16, tag="SG")
                        nc.scalar.activation(SG[:], G[:], AF.Sigmoid)
                        return SG

                    def stage_cell(dd, SG):
                        SGv = SG.rearrange("p (m s) -> p m s", m=NCH)
                        ihat = SGv[:, 0:4, :]
                        fhat = SGv[:, 4:8, :]
                        shat = SGv[:, 12:16, :]
                        t1 = rp[dd].tile([128, 16], BF16, tag="t1")
                        t2 = rp[dd].tile([128, 16], BF16, tag="t2")
                        cnext = rp[dd].tile([128, 16], BF16, tag="c")
                        cpv = cprev[dd].rearrange("p (k s) -> p k s", k=4)
                        t1v = t1.rearrange("p (k s) -> p k s", k=4)
                        t2v = t2.rearrange("p (k s) -> p k s", k=4)
                        cnv = cnext.rearrange("p (k s) -> p k s", k=4)
                        # c = fhat*c_prev + ihat*tanh(g)
                        #   = t1 + 2*((shat - 0.5)*ihat)   [tanh(g)=2s-1]
                        nc.vector.tensor_mul(t1v[:], fhat, cpv[:])
                        nc.vector.scalar_tensor_tensor(
                            out=t2v[:], in0=shat, scalar=0.5, in1=ihat,
                            op0=ALU.subtract, op1=ALU.mult)
                        nc.vector.scalar_tensor_tensor(
                            out=cnv[:], in0=t2v[:], scalar=2.0, in1=t1v[:],
                            op0=ALU.mult, op1=ALU.add)
                        cprev[dd] = cnext
                        return cnext

                    def stage_tanh(dd, cnext):
                        th = rp[dd].tile([128, 16], BF16, tag="th")
                        nc.scalar.activation(th[:], cnext[:], AF.Tanh)
                        return th

                    def stage_h(dd, u, SG, th):
                        SGv = SG.rearrange("p (m s) -> p m s", m=NCH)
                        ohat = SGv[:, 8:12, :]
                        hout = hv[:, dd, u, :, :]
                        thv = th.rearrange("p (k s) -> p k s", k=4)
                        nc.vector.tensor_mul(hout, ohat, thv[:])

                    # stage-major emission: each engine's stream is ordered
                    # by expected data-arrival time so the in-order engine
                    # FIFOs never head-block across the two chains.
                    for u in range(S):
                        G0 = stage_pe(0, u)
                        G1 = stage_pe(1, u)
                        SG0 = stage_sig(0, G0)
                        SG1 = stage_sig(1, G1)
                        c0 = stage_cell(0, SG0)
                        c1 = stage_cell(1, SG1)
                        th0 = stage_tanh(0, c0)
                        th1 = stage_tanh(1, c1)
                        stage_h(0, u, SG0, th0)
                        stage_h(1, u, SG1, th1)

                if lv == 2:
                    probe = pp.tile([1, NS], F32, tag="probe")
                    nc.vector.tensor_copy(probe[:], hsT[0:1, 0:NS])
                    nc.sync.dma_start(out=d_loss[:], in_=probe[:])

            # ---------- emissions ----------
            em_lin = pp.tile([T, N], F32, tag="em_lin")
            exp_em = pp.tile([T, N], F32, tag="exp_em")
            if lv >= 3:
                with tc.tile_pool(name="emc", bufs=1) as ec_, \
                     tc.tile_pool(name="ps_em", bufs=2, space="PSUM") as ps_em:
                    linb = ec_.tile([T, 1], F32, tag="linb")
                    nc.sync.dma_start(out=linb[:], in_=d["linb"][:])
                    hv_e = hsT.rearrange("p (a t k s) -> p a t k s",
                                         a=2, t=S + 1, k=HC)
                    for nb in range(2):
                        toff = nb * 128
                        # fwd: slot == position
                        pe_f = ps_em.tile([T, 512], F32, tag="pef")
                        for k in range(4):
                            rhs = hv_e[:, 0, toff:toff + 128, k:k + 1, :]
                            nc.tensor.matmul(
                                pe_f[:], linT[:, T * k: T * (k + 1)], rhs,
                                start=(k == 0), stop=(k == 3))
                        # bwd: position pos is at slot S-1-pos; slots
                        # [128-toff, 256-toff) cover positions
                        # [toff, toff+128) in DESCENDING order.
                        pe_b = ps_em.tile([T, 512], F32, tag="peb")
                        sb0 = 128 - toff
                        for k in range(4):
                            rhs = hv_e[:, 1, sb0:sb0 + 128, k:k + 1, :]
                            nc.tensor.matmul(
                                pe_b[:], linT[:, T * (4 + k): T * (5 + k)], rhs,
                                start=(k == 0), stop=(k == 3))
                        # em = pe_f + reversed(pe_b) (+ linb); pe_b goes
                        # through SBUF first (DVE TT can read only one PSUM
                        # operand, and the reversed AP stays on SBUF).
                        pb_s = ec_.tile([T, 512], F32, tag="pb_s")
                        nc.vector.tensor_copy(pb_s[:], pe_b[:])
                        pbv = pb_s.rearrange("p (w s) -> p w s", w=128)
                        em_nb = ec_.tile([T, 512], F32, tag="em_nb")
                        env = em_nb.rearrange("p (w s) -> p w s", w=128)
                        pfv = pe_f.rearrange("p (w s) -> p w s", w=128)
                        nc.vector.tensor_tensor(
                            out=env[:], in0=pfv[:], in1=pbv[:, ::-1, :],
                            op=ALU.add)
                        nc.vector.tensor_scalar_add(
                            em_lin[:, 512 * nb: 512 * (nb + 1)], em_nb[:],
                            linb[:])
                        nc.scalar.activation(exp_em[:, 512 * nb: 512 * (nb + 1)],
                                             em_nb[:], AF.Exp, bias=linb[:])
                if lv == 3:
                    probe = pp.tile([1, NS], F32, tag="probe")
                    nc.vector.tensor_copy(probe[:], em_lin[0:1, 0:NS])
                    nc.sync.dma_start(out=d_loss[:], in_=probe[:])

            # ---------- CRF ----------
            if lv >= 4:
                with tc.tile_pool(name="crf", bufs=1) as cp, \
                     tc.tile_pool(name="qs", bufs=3) as qp, \
                     tc.tile_pool(name="ps_q", bufs=2, space="PSUM") as ps_q:
                    trans_sb = cp.tile([T, T], F32, tag="trans")
                    stend = cp.tile([T, 2], F32, tag="stend")
                    estart = cp.tile([T, 1], F32, tag="estart")
                    eend = cp.tile([T, 1], F32, tag="eend")
                    nln45 = cp.tile([T, 1], F32, tag="nln45")
                    ones45 = cp.tile([T, 1], F32, tag="ones45")
                    oh = cp.tile([T, N], F32, tag="oh")
                    oh2 = cp.tile([T, N], F32, tag="oh2")
                    nc.sync.dma_start(out=trans_sb[:], in_=d["trans"][:])
                    nc.sync.dma_start(out=stend[:], in_=d["stend"][:])
                    nc.sync.dma_start(out=oh[:], in_=d["oh"][:])
                    nc.sync.dma_start(out=oh2[:], in_=d["oh2"][:])
                    nc.vector.memset(nln45[:], -LN45)
                    nc.vector.memset(ones45[:], 1.0)
                    nc.scalar.activation(estart[:], stend[:, 0:1], AF.Exp)
                    nc.scalar.activation(eend[:], stend[:, 1:2], AF.Exp)

                    # partition function via two-sided vector chains that
                    # meet at K=127:  Z = sum_i alpha_K(i) * beta_K(i).
                    K = 127
                    Epb = cp.tile([T, T], BF16, tag="Epb")
                    nc.scalar.activation(Epb[:], trans_sb[:], AF.Exp, bias=nln45[:])
                    EpbT = cp.tile([T, T], BF16, tag="EpbT")
                    with tc.tile_pool(name="ps_t", bufs=1, space="PSUM") as ps_t:
                        tpt = ps_t.tile([T, T], BF16, tag="tpt")
                        nc.tensor.transpose(tpt[:], Epb[:], idbf[0:T, 0:T])
                        nc.vector.tensor_copy(EpbT[:], tpt[:])

                    q = qp.tile([T, NS], BF16, tag="q")
                    nc.vector.tensor_scalar_mul(q[:], exp_em[:, 0:NS], estart[:])
                    bq0 = qp.tile([T, NS], BF16, tag="bq")
                    nc.vector.tensor_scalar_mul(
                        bq0[:], eend[:].to_broadcast([T, NS]), ones45[:])
                    bq = bq0                     # beta lives in PSUM after j=1
                    with tc.tile_pool(name="ps_b", bufs=2, space="PSUM") as ps_b:
                        for j in range(1, K + 1):
                            # alpha: t = j
                            sA = ps_q.tile([T, NS], F32, tag="sA")
                            nc.tensor.matmul(sA[:], Epb[:], q[:],
                                             start=True, stop=True)
                            qn = qp.tile([T, NS], BF16, tag="q")
                            nc.vector.tensor_mul(
                                qn[:], sA[:], exp_em[:, NS * j: NS * (j + 1)])
                            q = qn
                            # beta: t = 255 - j
                            t_ = S - 1 - j
                            wv = qp.tile([T, NS], BF16, tag="wv")
                            nc.vector.tensor_mul(
                                wv[:], bq[:],
                                exp_em[:, NS * (t_ + 1): NS * (t_ + 2)])
                            sB = ps_b.tile([T, NS], F32, tag="sB")
                            nc.tensor.matmul(sB[:], EpbT[:], wv[:],
                                             start=True, stop=True)
                            bq = sB
                        # one extra beta step so beta reaches position K
                        wv = qp.tile([T, NS], BF16, tag="wv")
                        nc.vector.tensor_mul(
                            wv[:], bq[:], exp_em[:, NS * (K + 1): NS * (K + 2)])
                        sB = ps_b.tile([T, NS], F32, tag="sB")
                        nc.tensor.matmul(sB[:], EpbT[:], wv[:],
                                         start=True, stop=True)
                        bqf = cp.tile([T, NS], F32, tag="bqf")
                        nc.vector.tensor_copy(bqf[:], sB[:])
                        bq = bqf
                    if lv == 4:
                        probe = pp.tile([1, NS], F32, tag="probe")
                        nc.vector.tensor_copy(probe[:], q[0:1, :])
                        nc.sync.dma_start(out=d_loss[:], in_=probe[:])

                    if lv >= 5:
                        w = cp.tile([T, NS], F32, tag="w")
                        logZ = cp.tile([1, NS], F32, tag="logZ")
                        em_h = cp.tile([1, 2 * NS], F32, tag="em_h")
                        tr_h = cp.tile([1, 2 * NS], F32, tag="tr_h")
                        em_sc = cp.tile([1, NS], F32, tag="em_sc")
                        tr_sc = cp.tile([1, NS], F32, tag="tr_sc")
                        sten_s = cp.tile([1, NS], F32, tag="sten_s")
                        nc.vector.tensor_mul(w[:], q[:], bq[:])
                        with tc.tile_pool(name="ps_f", bufs=1, space="PSUM") as ps_f:
                            sumw = ps_f.tile([1, NS], F32, tag="f1")
                            nc.tensor.matmul(sumw[:], ones45[:], w[:],
                                             start=True, stop=True)
                            nc.scalar.activation(logZ[:], sumw[:], AF.Ln)

                            S1 = cp.tile([T, N], F32, tag="S1")
                            nc.vector.tensor_mul(S1[:], em_lin[:], oh[:])
                            S2 = cp.tile([T, N], F32, tag="S2")
                            for ck in range(2):
                                sl = slice(512 * ck, 512 * (ck + 1))
                                s1p = ps_f.tile([1, 512], F32, tag="fbig")
                                nc.tensor.matmul(s1p[:], ones45[:], S1[:, sl],
                                                 start=True, stop=True)
                                nc.vector.tensor_reduce(
                                    em_h[:, NS * ck: NS * (ck + 1)],
                                    s1p.rearrange("p (t b) -> p b t", b=NS),
                                    axis=mybir.AxisListType.X, op=ALU.add)
                                Rp_ = ps_f.tile([T, 512], F32, tag="fR")
                                nc.tensor.matmul(Rp_[:], trans_sb[:], oh[:, sl],
                                                 start=True, stop=True)
                                nc.vector.tensor_mul(S2[:, sl], Rp_[:], oh2[:, sl])
                                s2p = ps_f.tile([1, 512], F32, tag="fbig2")
                                nc.tensor.matmul(s2p[:], ones45[:], S2[:, sl],
                                                 start=True, stop=True)
                                nc.vector.tensor_reduce(
                                    tr_h[:, NS * ck: NS * (ck + 1)],
                                    s2p.rearrange("p (t b) -> p b t", b=NS),
                                    axis=mybir.AxisListType.X, op=ALU.add)
                            nc.vector.tensor_add(em_sc[:], em_h[:, 0:NS],
                                                 em_h[:, NS:2 * NS])
                            nc.vector.tensor_add(tr_sc[:], tr_h[:, 0:NS],
                                                 tr_h[:, NS:2 * NS])

                            stp = cp.tile([T, NS], F32, tag="stp")
                            enp = cp.tile([T, NS], F32, tag="enp")
                            nc.vector.tensor_scalar_mul(stp[:], oh[:, 0:NS],
                                                        stend[:, 0:1])
                            nc.vector.tensor_scalar_mul(enp[:], oh[:, N - NS:N],
                                                        stend[:, 1:2])
                            sten = ps_f.tile([1, NS], F32, tag="f2")
                            nc.tensor.matmul(sten[:], ones45[:], stp[:],
                                             start=True, stop=False)
                            nc.tensor.matmul(sten[:], ones45[:], enp[:],
                                             start=False, stop=True)
                            nc.vector.tensor_copy(sten_s[:], sten[:])

                        sc1 = cp.tile([1, NS], F32, tag="sc1")
                        sc2 = cp.tile([1, NS], F32, tag="sc2")
                        lossa = cp.tile([1, NS], F32, tag="lossa")
                        lossb = cp.tile([1, NS], F32, tag="lossb")
                        nc.vector.tensor_add(sc1[:], em_sc[:], tr_sc[:])
                        nc.vector.tensor_add(sc2[:], sc1[:], sten_s[:])
                        nc.vector.tensor_tensor(out=lossa[:], in0=logZ[:],
                                                in1=sc2[:], op=ALU.subtract)
                        nc.scalar.activation(lossb[:], lossa[:], AF.Copy,
                                             bias=(S - 1) * LN45)
                        nc.sync.dma_start(out=d_loss[:], in_=lossb[:])

    nc.finalize()
    return nc


def _pack_wT(w, kchunks):
    # w: [M_out rows (gate units, reordered), K] ->
    # [128, (nm*kchunks)*128] tiles: tile (m*kchunks+ec) = w[mU, ecK].T
    M, K = w.shape
    nm = M // 128
    assert K == 128 * kchunks
    tiles = []
    for m in range(nm):
        for ec in range(kchunks):
            blk = w[m * 128:(m + 1) * 128, ec * 128:(ec + 1) * 128]
            tiles.append(np.ascontiguousarray(blk.T))
    return np.concatenate(tiles, axis=1)


def _perm_gates_ifog(w):
    # torch gate order i,f,g,o (blocks of H) -> our chunk order i,f,o,g;
    # g rows scaled by 2 for the sigma-trick (tanh(g) = 2*sigma(2g) - 1)
    i, f, g, o = np.split(w, 4, axis=0)
    return np.concatenate([i, f, o, 2.0 * g], axis=0)


def prepare_in_maps(**inputs):
    x = np.asarray(inputs["x"]).astype(np.int32)          # [32, 256]
    tags = np.asarray(inputs["tags"]).astype(np.int32)
    emb = np.asarray(inputs["emb"], dtype=np.float32)
    lin_w = np.asarray(inputs["lin_w"], dtype=np.float32)
    lin_b = np.asarray(inputs["lin_b"], dtype=np.float32)
    start_t = np.asarray(inputs["start_t"], dtype=np.float32)
    end_t = np.asarray(inputs["end_t"], dtype=np.float32)
    trans = np.asarray(inputs["trans"], dtype=np.float32)

    wihp = {0: _perm_gates_ifog(np.asarray(inputs["w_ih_f"], np.float32)),
            1: _perm_gates_ifog(np.asarray(inputs["w_ih_b"], np.float32))}
    whhp = {0: _perm_gates_ifog(np.asarray(inputs["w_hh_f"], np.float32)),
            1: _perm_gates_ifog(np.asarray(inputs["w_hh_b"], np.float32))}
    bp = {0: _perm_gates_ifog(np.asarray(inputs["b_f"], np.float32).reshape(-1, 1)),
          1: _perm_gates_ifog(np.asarray(inputs["b_b"], np.float32).reshape(-1, 1))}

    wih_t = {dd: _pack_wT(wihp[dd], 2).astype(ml_dtypes.float8_e4m3) for dd in (0, 1)}
    whh_t = {dd: _pack_wT(whhp[dd], 4).astype(ml_dtypes.float8_e4m3) for dd in (0, 1)}

    # biasbc [128, 128]: col = d*64 + m*4 + s -> b_d[m*128 + p]
    biasbc = np.zeros((128, 128), np.float32)
    for dd in (0, 1):
        for m in range(16):
            col = bp[dd][m * 128:(m + 1) * 128, 0]
            for s in range(4):
                biasbc[:, dd * 64 + 4 * m + s] = col
    biasbc = biasbc.astype(ml_dtypes.bfloat16)

    # linT [128, 8*T]: tile kc = lin_w[:, kc*128:(kc+1)*128].T (fwd 0-3, bwd 4-7)
    lin_tiles = [np.ascontiguousarray(lin_w[:, kc * 128:(kc + 1) * 128].T)
                 for kc in range(8)]
    linT = np.concatenate(lin_tiles, axis=1).astype(ml_dtypes.bfloat16)

    id128 = np.eye(128, dtype=np.float32)

    in_maps = []
    for core in range(8):
        seqs = slice(4 * core, 4 * core + 4)
        xs = x[seqs]                                      # [4, 256]
        # xidx [128, 8]: col b, row r -> x[s=(r%4), t=(128b+r)//4]
        nflat = xs.T.reshape(-1)                          # n = 4t+s
        xidx = np.ascontiguousarray(nflat.reshape(8, 128).T).astype(np.int32)

        tg = tags[seqs]                                   # [4, 256]
        oh = np.zeros((T, N), np.float32)
        oh[tg.T.reshape(-1), np.arange(N)] = 1.0
        oh2 = np.zeros((T, N), np.float32)
        oh2[:, 0:N - NS] = oh[:, NS:N]

        in_maps.append({
            "emb": emb.astype(ml_dtypes.bfloat16),
            "xidx": xidx,
            "wihf": wih_t[0], "wihb": wih_t[1],
            "whhf": whh_t[0], "whhb": whh_t[1],
            "biasbc": biasbc,
            "linT": linT,
            "linb": lin_b.reshape(T, 1),
            "id128": id128,
            "idbf": np.eye(128, dtype=ml_dtypes.bfloat16),
            "trans": trans,
            "stend": np.stack([start_t, end_t], axis=1),
            "oh": oh,
            "oh2": oh2,
        })
    return in_maps


def get_nc():
    if "nc" not in _cached:
        _cached["nc"] = _build()
    return _cached["nc"]


def kernel(**inputs):
    in_maps = prepare_in_maps(**inputs)
    res = run_bass_kernel_spmd(get_nc(), in_maps, core_ids=list(range(8)))
    total = np.float64(0.0)
    for core in range(8):
        total += np.float64(res.results[core]["loss"]).sum()
    return np.float32(total / 32.0)
